# revision 35
# baseline (speedup 1.0000x reference)
"""BarlowTwins-style loss kernel for Trainium2 (raw Bass), 8-core SPMD.

Math: the reference materializes a (B, D, D) per-sample cross-correlation
tensor, but the loss algebraically reduces to O(B*D) work.  With
z1n/z2n the per-dim (batch-)normalized inputs and per-sample b:
    w    = z1n[b,:] * z2n[b,:]
    R    = sum(w);  P = sum(w^2);  Sa = sum(z1n^2);  Sv = sum(z2n^2)
    a    = z1n[b,b];  v = z2n[b,b];  d = a*v;  g2 = (d-1)^2
    u    = (a*z2n[b,:] - 1)^2;  Q = sum(u^2);  (g2 == u[b])
    T    = P - 2R + D                  # sum((w-1)^2)
    on   = T - g2 + (g2-1)^2
    off  = (Sa - a^2)*Sv - P + d^2 + Q - g2^2
    loss = on + 0.005*off

Sharding: data-parallel over batch.  Every core loads the full z1/z2 to
compute per-dim column sums/sumsq locally (cheaper than an all-reduce at
this size), then computes the loss for its own 16 samples using a
rearranged layout [128 partitions = (sample, col-chunk), 128 free] so all
row-reductions run at full partition occupancy.

Written in raw Bass (explicit semaphores): the walrus build in this
container only supports a single sync-wait per instruction, which the
Tile framework's exit sequence violates; standalone wait_ge instructions
compose freely.
"""

import sys
from contextlib import ExitStack

import numpy as np

for _p in ("/opt/trn_rl_repo",):
    if _p not in sys.path:
        sys.path.append(_p)

import concourse.bass as bass
import concourse.mybir as mybir
from concourse.bass_utils import run_bass_kernel_spmd

B, D = 128, 1024
NCORES = 8
SPC = B // NCORES  # 16 samples per core
KCH = D // 128     # 8 column chunks of 128
LAM = 0.005

FP = mybir.dt.float32
BF = mybir.dt.bfloat16
AF = mybir.ActivationFunctionType
AL = mybir.AluOpType

# consts column layout
C_AMASK = 0
C_EXPAND = 128
C_Z1R = 256
C_Z2R = 384
C_GSEL = 512
C_SELZ1 = 528
C_SELZ2 = 656
C_TOTAL = 784


def build_program():
    nc = bass.Bass("TRN2", debug=False, num_devices=NCORES,
                   detect_race_conditions=False)

    z1_d = nc.dram_tensor("z1", [B, D], FP, kind="ExternalInput")
    z2_d = nc.dram_tensor("z2", [B, D], FP, kind="ExternalInput")
    consts_d = nc.dram_tensor("consts", [128, C_TOTAL], FP, kind="ExternalInput")
    loss_d = nc.dram_tensor("loss", [SPC, 1], FP, kind="ExternalOutput")

    ctx = ExitStack()
    with ctx:
        sem = {n: ctx.enter_context(nc.semaphore(n)) for n in
               ["sca", "dz0", "dz1", "dz2", "dz3", "sscat", "qscat", "dout",
                "spe", "sv", "sa", "sg"]}

        def sb(name, shape, dtype=FP):
            return ctx.enter_context(nc.sbuf_tensor(name, shape, dtype))

        ca = sb("ca", [128, C_TOTAL])
        z = sb("z", [128, 2 * D])
        zb = sb("zb", [128, 2 * D], BF)
        sqb = sb("sqb", [128, 2 * D], BF)
        sumrow = sb("sumrow", [1, 2 * D])
        qrow = sb("qrow", [1, 2 * D])
        sum16 = sb("sum16", [16, 128])
        q16s = sb("q16s", [16, 128])
        q127 = sb("q127", [16, 128])
        t1 = sb("t1", [16, 128])
        var16 = sb("var16", [16, 128])
        std16 = sb("std16", [16, 128])
        A_z = sb("A_z", [16, 128])
        C_z = sb("C_z", [16, 128])
        tn1 = sb("tn1", [128, 128])
        z1n = sb("z1n", [128, 128])
        tn2 = sb("tn2", [128, 128])
        z2n = sb("z2n", [128, 128])
        w = sb("w", [128, 128])
        u = sb("u", [128, 128])
        junkP = sb("junkP", [128, 128])
        junkQ = sb("junkQ", [128, 128])
        junkA = sb("junkA", [128, 128])
        junkV = sb("junkV", [128, 128])
        junkG = sb("junkG", [128, 128])
        junkA2 = sb("junkA2", [128, 128])
        junkV2 = sb("junkV2", [128, 128])
        negone = sb("negone", [128, 1])
        ones_b = sb("ones_b", [128, 1], BF)
        acol = sb("acol", [128, 1])
        vcol = sb("vcol", [128, 1])
        a_sb = sb("a_sb", [128, 1])
        colsD = sb("colsD", [128, 8])
        q16sb = sb("q16sb", [16, 8])
        fin = sb("fin", [16, 13])
        loss16 = sb("loss16", [16, 1])

        # PSUM: 4 stat banks (reused by blocks 2,3) + broadcast bank +
        # a-expand bank + group-reduce bank = 7 of 8 banks.
        ps = [ctx.enter_context(nc.psum_tensor(f"ps{i}", [1, 512], FP))
              for i in range(4)]
        ps_s_t = [ps[0], ps[2], ps[0], ps[2]]
        ps_q_t = [ps[1], ps[3], ps[1], ps[3]]
        psBC = ctx.enter_context(nc.psum_tensor("psBC", [128, 512], FP))
        a_ps = ctx.enter_context(nc.psum_tensor("a_ps", [128, 1], FP))
        q16 = ctx.enter_context(nc.psum_tensor("q16", [16, 8], FP))

        psA1 = psBC[:, 0:128]
        psC1 = psBC[:, 128:256]
        psA2 = psBC[:, 256:384]
        psC2 = psBC[:, 384:512]

        amask = ca[:, C_AMASK:C_AMASK + 128]
        expand = ca[:, C_EXPAND:C_EXPAND + 128]
        z1r = ca[:, C_Z1R:C_Z1R + 128]
        z2r = ca[:, C_Z2R:C_Z2R + 128]
        gsel = ca[:, C_GSEL:C_GSEL + 16]
        selz1 = ca[0:16, C_SELZ1:C_SELZ1 + 128]
        selz2 = ca[0:16, C_SELZ2:C_SELZ2 + 128]

        blksl = [slice(i * 512, (i + 1) * 512) for i in range(4)]

        with nc.Block() as block:

            @block.sync
            def _(sync):
                sync.dma_start(ca[:], consts_d[:]).then_inc(sem["sca"], 16)
                sync.dma_start(z[:, blksl[0]], z1_d[:, 0:512]).then_inc(sem["dz0"], 16)
                sync.dma_start(z[:, blksl[1]], z1_d[:, 512:1024]).then_inc(sem["dz1"], 16)
                sync.dma_start(z[:, blksl[2]], z2_d[:, 0:512]).then_inc(sem["dz2"], 16)
                sync.dma_start(z[:, blksl[3]], z2_d[:, 512:1024]).then_inc(sem["dz3"], 16)
                # scatter stat rows into [16,128] once rows are complete
                sync.wait_ge(sem["sv"], 8)
                sync.dma_start(sum16[:], sumrow[:]).then_inc(sem["sscat"], 16)
                sync.wait_ge(sem["sa"], 6)
                sync.dma_start(q16s[:], qrow[:]).then_inc(sem["qscat"], 16)
                # output
                sync.wait_ge(sem["sv"], 47)
                sync.dma_start(loss_d[:], loss16[:]).then_inc(sem["dout"], 16)
                sync.wait_ge(sem["dout"], 16)

            @block.gpsimd
            def _(gp):
                gp.wait_ge(sem["dz0"], 16)
                gp.tensor_copy(zb[:, blksl[0]], z[:, blksl[0]]).then_inc(sem["sg"])       # 1
                gp.tensor_tensor(sqb[:, blksl[0]], z[:, blksl[0]], z[:, blksl[0]],
                                 AL.mult).then_inc(sem["sg"])                             # 2
                gp.wait_ge(sem["dz1"], 16)
                gp.tensor_copy(zb[:, blksl[1]], z[:, blksl[1]]).then_inc(sem["sg"])       # 3
                gp.tensor_tensor(sqb[:, blksl[1]], z[:, blksl[1]], z[:, blksl[1]],
                                 AL.mult).then_inc(sem["sg"])                             # 4

            @block.scalar
            def _(act):
                act.wait_ge(sem["dz3"], 16)
                act.copy(zb[:, blksl[3]], z[:, blksl[3]]).then_inc(sem["sa"])             # 1
                act.square(sqb[:, blksl[3]], z[:, blksl[3]]).then_inc(sem["sa"])          # 2
                for i in range(4):                                                        # 3-6
                    act.wait_ge(sem["spe"], 2 * i + 2)
                    act.copy(qrow[:, blksl[i]], ps_q_t[i][:]).then_inc(sem["sa"])
                act.wait_ge(sem["sv"], 11)
                act.activation(std16[:], var16[:], AF.Sqrt).then_inc(sem["sa"])           # 7
                act.wait_ge(sem["sv"], 24)
                act.activation(u[:], z2n[:], AF.Square, bias=negone[:],
                               scale=a_sb[:]).then_inc(sem["sa"])                         # 8
                act.wait_ge(sem["spe"], 14)
                act.copy(q16sb[:], q16[:]).then_inc(sem["sa"])                            # 9

            @block.vector
            def _(dve):
                dve.memset(negone[:], -1.0).then_inc(sem["sv"])                           # 1
                dve.memset(ones_b[:], 1.0).then_inc(sem["sv"])                            # 2
                dve.wait_ge(sem["dz2"], 16)
                dve.tensor_copy(zb[:, blksl[2]], z[:, blksl[2]]).then_inc(sem["sv"])      # 3
                dve.tensor_tensor(sqb[:, blksl[2]], z[:, blksl[2]],
                                  z[:, blksl[2]], AL.mult).then_inc(sem["sv"])            # 4
                for i in range(4):                                                        # 5-8
                    dve.wait_ge(sem["spe"], 2 * i + 1)
                    dve.tensor_copy(sumrow[:, blksl[i]], ps_s_t[i][:]).then_inc(sem["sv"])
                dve.wait_ge(sem["qscat"], 16)
                dve.tensor_scalar_mul(q127[:], q16s[:],
                                      1.0 / (B - 1.0)).then_inc(sem["sv"])                # 9
                dve.wait_ge(sem["sscat"], 16)
                dve.scalar_tensor_tensor(
                    t1[:], sum16[:], 1.0 / (B * (B - 1.0)), sum16[:],
                    op0=AL.mult, op1=AL.mult).then_inc(sem["sv"])                         # 10
                dve.tensor_tensor(var16[:], q127[:], t1[:],
                                  AL.subtract).then_inc(sem["sv"])                        # 11
                dve.wait_ge(sem["sa"], 7)
                dve.reciprocal(A_z[:], std16[:]).then_inc(sem["sv"])                      # 12
                dve.scalar_tensor_tensor(
                    C_z[:], sum16[:], 1.0 / B, A_z[:],
                    op0=AL.mult, op1=AL.mult).then_inc(sem["sv"])                         # 13
                # normalize (reads ca and PSUM broadcasts).  Wait for ALL
                # four broadcast matmuls first: concurrent PE-write + DVE-read
                # of the same PSUM bank is fatal on HW.
                dve.wait_ge(sem["sca"], 16)
                dve.wait_ge(sem["spe"], 12)
                dve.tensor_tensor(tn1[:], z1r, psA1, AL.mult).then_inc(sem["sv"])         # 14
                dve.tensor_tensor(z1n[:], tn1[:], psC1, AL.subtract).then_inc(sem["sv"])  # 15
                dve.tensor_tensor(tn2[:], z2r, psA2, AL.mult).then_inc(sem["sv"])         # 16
                dve.tensor_tensor(z2n[:], tn2[:], psC2, AL.subtract).then_inc(sem["sv"])  # 17
                dve.scalar_tensor_tensor(
                    w[:], z1n[:], 1.0, z2n[:], op0=AL.bypass, op1=AL.mult,
                    accum_out=colsD[:, 0:1]).then_inc(sem["sv"])                          # 18
                dve.scalar_tensor_tensor(
                    junkP[:], w[:], 1.0, w[:], op0=AL.bypass, op1=AL.mult,
                    accum_out=colsD[:, 1:2]).then_inc(sem["sv"])                          # 19
                dve.scalar_tensor_tensor(
                    junkA2[:], z1n[:], 1.0, z1n[:], op0=AL.bypass, op1=AL.mult,
                    accum_out=colsD[:, 6:7]).then_inc(sem["sv"])                          # 20 Sa
                dve.scalar_tensor_tensor(
                    junkV2[:], z2n[:], 1.0, z2n[:], op0=AL.bypass, op1=AL.mult,
                    accum_out=colsD[:, 7:8]).then_inc(sem["sv"])                          # 21 Sv
                dve.scalar_tensor_tensor(
                    junkA[:], z1n[:], 1.0, amask, op0=AL.bypass, op1=AL.mult,
                    accum_out=acol[:]).then_inc(sem["sv"])                                # 20
                dve.scalar_tensor_tensor(
                    junkV[:], z2n[:], 1.0, amask, op0=AL.bypass, op1=AL.mult,
                    accum_out=vcol[:]).then_inc(sem["sv"])                                # 23
                # NOTE: reads of accum_out results need pipeline distance from
                # their producer on the same engine (the accumulator write
                # lands late); d/a2 are deferred below.
                dve.wait_ge(sem["spe"], 13)
                dve.tensor_copy(a_sb[:], a_ps[:]).then_inc(sem["sv"])                     # 24
                dve.wait_ge(sem["sa"], 8)
                dve.scalar_tensor_tensor(
                    junkQ[:], u[:], 1.0, u[:], op0=AL.bypass, op1=AL.mult,
                    accum_out=colsD[:, 2:3]).then_inc(sem["sv"])                          # 25
                dve.scalar_tensor_tensor(
                    junkG[:], u[:], 1.0, amask, op0=AL.bypass, op1=AL.mult,
                    accum_out=colsD[:, 3:4]).then_inc(sem["sv"])                          # 26
                dve.tensor_tensor(colsD[:, 4:5], acol[:], vcol[:],
                                  AL.mult).then_inc(sem["sv"])                            # 27
                dve.tensor_tensor(colsD[:, 5:6], acol[:], acol[:],
                                  AL.mult).then_inc(sem["sv"])                            # 28
                # ---- finals on [16,1] ----
                R_ = q16sb[:, 0:1]
                P_ = q16sb[:, 1:2]
                Q_ = q16sb[:, 2:3]
                gd_ = q16sb[:, 3:4]
                d_ = q16sb[:, 4:5]
                a2_ = q16sb[:, 5:6]
                Sa_ = q16sb[:, 6:7]
                Sv_ = q16sb[:, 7:8]
                d2 = fin[:, 0:1]
                g4 = fin[:, 1:2]
                h = fin[:, 2:3]
                Tp = fin[:, 3:4]
                on1 = fin[:, 4:5]
                on2 = fin[:, 5:6]
                e1 = fin[:, 6:7]
                f1 = fin[:, 7:8]
                f2 = fin[:, 8:9]
                f3 = fin[:, 9:10]
                f4 = fin[:, 10:11]
                off = fin[:, 11:12]
                hm = fin[:, 12:13]
                dve.wait_ge(sem["sa"], 9)
                # Final chain: same-engine RAW on tiny [16,1] ops needs >=2
                # intervening instructions or a drain (writes land late).
                dve.tensor_tensor(d2, d_, d_, AL.mult).then_inc(sem["sv"])       # 29
                dve.tensor_tensor(g4, gd_, gd_, AL.mult).then_inc(sem["sv"])     # 30
                dve.tensor_scalar_add(hm, gd_, -1.0).then_inc(sem["sv"])         # 31
                dve.scalar_tensor_tensor(
                    Tp, R_, -2.0, P_, op0=AL.mult, op1=AL.add).then_inc(sem["sv"])  # 32
                dve.tensor_tensor(e1, Sa_, a2_, AL.subtract).then_inc(sem["sv"])    # 33
                dve.tensor_tensor(h, hm, hm, AL.mult).then_inc(sem["sv"])        # 34 [hm gap 2]
                dve.tensor_tensor(f2, d2, Q_, AL.add).then_inc(sem["sv"])        # 35 [d2 gap 5]
                dve.tensor_tensor(on1, Tp, gd_, AL.subtract).then_inc(sem["sv"])  # 36 [Tp gap 3]
                dve.tensor_tensor(f1, e1, Sv_, AL.mult).then_inc(sem["sv"])      # 37 [e1 gap 3]
                dve.tensor_tensor(f3, f2, P_, AL.subtract).then_inc(sem["sv"])   # 38 [f2 gap 2]
                dve.tensor_tensor(on2, on1, h, AL.add).then_inc(sem["sv"])       # 39 [on1 g2, h g4]
                dve.drain().then_inc(sem["sv"])                                  # 40
                dve.tensor_tensor(f4, f3, g4, AL.subtract).then_inc(sem["sv"])   # 41
                dve.drain().then_inc(sem["sv"])                                  # 42
                dve.tensor_tensor(off, f4, f1, AL.add).then_inc(sem["sv"])       # 43
                dve.drain().then_inc(sem["sv"])                                  # 44
                dve.scalar_tensor_tensor(
                    loss16[:], off, LAM, on2,
                    op0=AL.mult, op1=AL.add).then_inc(sem["sv"])                 # 45
                dve.drain().then_inc(sem["sv"])                                  # 46
                dve.tensor_scalar_add(loss16[:], loss16[:],
                                      float(D)).then_inc(sem["sv"])              # 47

            @block.tensor
            def _(pe):
                pe.wait_ge(sem["sv"], 2)  # ones_b ready
                # column sums: mm ticks 1-8
                # blk0 (GPSIMD-produced)
                pe.wait_ge(sem["sg"], 1)
                pe.matmul(ps_s_t[0][:], ones_b[:], zb[:, blksl[0]],
                          start=True, stop=True).then_inc(sem["spe"])                     # 1
                pe.wait_ge(sem["sg"], 2)
                pe.matmul(ps_q_t[0][:], ones_b[:], sqb[:, blksl[0]],
                          start=True, stop=True).then_inc(sem["spe"])                     # 2
                # blk1 (GPSIMD-produced)
                pe.wait_ge(sem["sg"], 3)
                pe.matmul(ps_s_t[1][:], ones_b[:], zb[:, blksl[1]],
                          start=True, stop=True).then_inc(sem["spe"])                     # 3
                pe.wait_ge(sem["sg"], 4)
                pe.matmul(ps_q_t[1][:], ones_b[:], sqb[:, blksl[1]],
                          start=True, stop=True).then_inc(sem["spe"])                     # 4
                # blk2 (DVE-produced; bank reuse waits for row copies)
                pe.wait_ge(sem["sv"], 5)  # covers zb2 (3) + s0 row copy (5)
                pe.matmul(ps_s_t[2][:], ones_b[:], zb[:, blksl[2]], start=True,
                          stop=True, skip_group_check=True).then_inc(sem["spe"])          # 5
                pe.wait_ge(sem["sa"], 3)  # q0 row copy
                pe.matmul(ps_q_t[2][:], ones_b[:], sqb[:, blksl[2]], start=True,
                          stop=True, skip_group_check=True).then_inc(sem["spe"])          # 6
                # blk3 (ACT-produced)
                pe.wait_ge(sem["sv"], 6)  # s1 row copy
                pe.wait_ge(sem["sa"], 1)  # zb3
                pe.matmul(ps_s_t[3][:], ones_b[:], zb[:, blksl[3]], start=True,
                          stop=True, skip_group_check=True).then_inc(sem["spe"])          # 7
                pe.wait_ge(sem["sa"], 4)  # q1 row copy (covers sqb3 at 2)
                pe.matmul(ps_q_t[3][:], ones_b[:], sqb[:, blksl[3]], start=True,
                          stop=True, skip_group_check=True).then_inc(sem["spe"])          # 8
                # broadcasts: mm ticks 9-12
                pe.wait_ge(sem["sca"], 16)
                pe.wait_ge(sem["sv"], 12)
                pe.matmul(psA1, selz1, A_z[:], start=True,
                          stop=True).then_inc(sem["spe"])                                 # 9
                pe.wait_ge(sem["sv"], 13)
                pe.matmul(psC1, selz1, C_z[:], start=True, stop=True,
                          skip_group_check=True).then_inc(sem["spe"])                     # 10
                pe.matmul(psA2, selz2, A_z[:], start=True, stop=True,
                          skip_group_check=True).then_inc(sem["spe"])                     # 11
                pe.matmul(psC2, selz2, C_z[:], start=True, stop=True,
                          skip_group_check=True).then_inc(sem["spe"])                     # 12
                # expand a: mm tick 13
                pe.wait_ge(sem["sv"], 22)
                pe.matmul(a_ps[:], expand, acol[:], start=True,
                          stop=True).then_inc(sem["spe"])                                 # 13
                # group reduce: mm ticks 14, 15
                pe.wait_ge(sem["sv"], 28)
                pe.matmul(q16[:], gsel, colsD[:], start=True,
                          stop=True).then_inc(sem["spe"])                                 # 14

    return nc


def _host_inputs(z1, z2):
    """Per-core input maps (sharding glue)."""
    z1 = np.ascontiguousarray(z1, np.float32)
    z2 = np.ascontiguousarray(z2, np.float32)

    base = np.zeros((128, C_TOTAL), np.float32)
    for m in range(128):
        base[8 * (m // 8), C_EXPAND + m] = 1.0   # expand
        base[m, C_GSEL + m // 8] = 1.0           # gsel
        base[m % 8, C_SELZ1 + m] = 1.0           # selz1
        base[8 + m % 8, C_SELZ2 + m] = 1.0       # selz2

    in_maps = []
    for c in range(NCORES):
        rows = slice(c * SPC, (c + 1) * SPC)
        consts = base.copy()
        consts[:, C_Z1R:C_Z1R + 128] = \
            z1[rows].reshape(SPC, KCH, 128).reshape(128, 128)
        consts[:, C_Z2R:C_Z2R + 128] = \
            z2[rows].reshape(SPC, KCH, 128).reshape(128, 128)
        for s in range(SPC):
            consts[s * 8, C_AMASK + c * SPC + s] = 1.0
        in_maps.append({
            "z1": z1, "z2": z2,
            "consts": np.ascontiguousarray(consts),
        })
    return in_maps


_cached_nc = None


def run(z1, z2, trace=False, **kwargs):
    global _cached_nc
    if _cached_nc is None:
        _cached_nc = build_program()
    in_maps = _host_inputs(z1, z2)
    res = run_bass_kernel_spmd(
        _cached_nc, in_maps, core_ids=list(range(NCORES)), trace=trace, **kwargs)
    out = np.concatenate([res.results[c]["loss"][:, 0] for c in range(NCORES)])
    return out.astype(np.float32), res


def kernel(z1, z2):
    out, _ = run(z1, z2, trace=False)
    return out


# revision 37
# speedup vs baseline: 1.1401x; 1.1401x over previous
"""BarlowTwins-style loss kernel for Trainium2 (raw Bass), 8-core SPMD.

Math: the reference materializes a (B, D, D) per-sample cross-correlation
tensor, but the loss algebraically reduces to O(B*D) work.  With
z1n/z2n the per-dim (batch-)normalized inputs and per-sample b:
    w    = z1n[b,:] * z2n[b,:]
    R    = sum(w);  P = sum(w^2);  Sa = sum(z1n^2);  Sv = sum(z2n^2)
    a    = z1n[b,b];  v = z2n[b,b];  d = a*v;  g2 = (d-1)^2
    u    = (a*z2n[b,:] - 1)^2;  Q = sum(u^2);  (g2 == u[b])
    T    = P - 2R + D                  # sum((w-1)^2)
    on   = T - g2 + (g2-1)^2
    off  = (Sa - a^2)*Sv - P + d^2 + Q - g2^2
    loss = on + 0.005*off

Sharding: data-parallel over batch.  Every core loads the full z1/z2 to
compute per-dim column sums/sumsq locally (cheaper than an all-reduce at
this size), then computes the loss for its own 16 samples using a
rearranged layout [128 partitions = (sample, col-chunk), 128 free] so all
row-reductions run at full partition occupancy.

Written in raw Bass (explicit semaphores): the walrus build in this
container only supports a single sync-wait per instruction, which the
Tile framework's exit sequence violates; standalone wait_ge instructions
compose freely.
"""

import sys
from contextlib import ExitStack

import numpy as np

for _p in ("/opt/trn_rl_repo",):
    if _p not in sys.path:
        sys.path.append(_p)

import concourse.bass as bass
import concourse.mybir as mybir
from concourse.bass_utils import run_bass_kernel_spmd

B, D = 128, 1024
NCORES = 8
SPC = B // NCORES  # 16 samples per core
KCH = D // 128     # 8 column chunks of 128
LAM = 0.005

FP = mybir.dt.float32
BF = mybir.dt.bfloat16
AF = mybir.ActivationFunctionType
AL = mybir.AluOpType

# consts column layout
C_AMASK = 0
C_EXPAND = 128
C_Z1R = 256
C_Z2R = 384
C_GSEL = 512
C_SELZ1 = 528
C_SELZ2 = 656
C_TOTAL = 784


def build_program():
    nc = bass.Bass("TRN2", debug=False, num_devices=NCORES,
                   detect_race_conditions=False)

    z1_d = nc.dram_tensor("z1", [B, D], FP, kind="ExternalInput")
    z2_d = nc.dram_tensor("z2", [B, D], FP, kind="ExternalInput")
    consts_d = nc.dram_tensor("consts", [128, C_TOTAL], FP, kind="ExternalInput")
    loss_d = nc.dram_tensor("loss", [SPC, 1], FP, kind="ExternalOutput")

    ctx = ExitStack()
    with ctx:
        sem = {n: ctx.enter_context(nc.semaphore(n)) for n in
               ["sca", "dz0", "dz1", "dz2", "dz3", "sscat", "qscat", "dout",
                "spe", "sv", "sa", "sg"]}

        def sb(name, shape, dtype=FP):
            return ctx.enter_context(nc.sbuf_tensor(name, shape, dtype))

        ca = sb("ca", [128, C_TOTAL])
        z = sb("z", [128, 2 * D])
        zb = sb("zb", [128, 2 * D], BF)
        sqb = sb("sqb", [128, 2 * D], BF)
        sumrow = sb("sumrow", [1, 2 * D])
        qrow = sb("qrow", [1, 2 * D])
        sum16 = sb("sum16", [16, 128])
        q16s = sb("q16s", [16, 128])
        q127 = sb("q127", [16, 128])
        t1 = sb("t1", [16, 128])
        var16 = sb("var16", [16, 128])
        std16 = sb("std16", [16, 128])
        A_z = sb("A_z", [16, 128])
        C_z = sb("C_z", [16, 128])
        tn1 = sb("tn1", [128, 128])
        z1n = sb("z1n", [128, 128])
        tn2 = sb("tn2", [128, 128])
        z2n = sb("z2n", [128, 128])
        w = sb("w", [128, 128])
        u = sb("u", [128, 128])
        junkP = sb("junkP", [128, 128])
        junkQ = sb("junkQ", [128, 128])
        junkA = sb("junkA", [128, 128])
        junkV = sb("junkV", [128, 128])
        junkG = sb("junkG", [128, 128])
        junkA2 = sb("junkA2", [128, 128])
        junkV2 = sb("junkV2", [128, 128])
        negone = sb("negone", [128, 1])
        ones_b = sb("ones_b", [128, 1], BF)
        acol = sb("acol", [128, 1])
        vcol = sb("vcol", [128, 1])
        a_sb = sb("a_sb", [128, 1])
        colsD = sb("colsD", [128, 8])
        q16sb = sb("q16sb", [16, 8])
        fin = sb("fin", [16, 14])
        loss16 = sb("loss16", [16, 1])

        # PSUM: 4 stat banks (reused by blocks 2,3) + broadcast bank +
        # a-expand bank + group-reduce bank = 7 of 8 banks.
        ps = [ctx.enter_context(nc.psum_tensor(f"ps{i}", [1, 512], FP))
              for i in range(4)]
        ps_s_t = [ps[0], ps[2], ps[0], ps[2]]
        ps_q_t = [ps[1], ps[3], ps[1], ps[3]]
        psBC = ctx.enter_context(nc.psum_tensor("psBC", [128, 512], FP))
        a_ps = ctx.enter_context(nc.psum_tensor("a_ps", [128, 1], FP))
        q16 = ctx.enter_context(nc.psum_tensor("q16", [16, 8], FP))

        psA1 = psBC[:, 0:128]
        psC1 = psBC[:, 128:256]
        psA2 = psBC[:, 256:384]
        psC2 = psBC[:, 384:512]

        amask = ca[:, C_AMASK:C_AMASK + 128]
        expand = ca[:, C_EXPAND:C_EXPAND + 128]
        z1r = ca[:, C_Z1R:C_Z1R + 128]
        z2r = ca[:, C_Z2R:C_Z2R + 128]
        gsel = ca[:, C_GSEL:C_GSEL + 16]
        selz1 = ca[0:16, C_SELZ1:C_SELZ1 + 128]
        selz2 = ca[0:16, C_SELZ2:C_SELZ2 + 128]

        blksl = [slice(i * 512, (i + 1) * 512) for i in range(4)]

        with nc.Block() as block:

            @block.sync
            def _(sync):
                # z blocks first (they gate compute); consts last
                sync.dma_start(z[:, blksl[0]], z1_d[:, 0:512]).then_inc(sem["dz0"], 16)
                sync.dma_start(z[:, blksl[1]], z1_d[:, 512:1024]).then_inc(sem["dz1"], 16)
                sync.dma_start(z[:, blksl[2]], z2_d[:, 0:512]).then_inc(sem["dz2"], 16)
                sync.dma_start(z[:, blksl[3]], z2_d[:, 512:1024]).then_inc(sem["dz3"], 16)
                sync.dma_start(ca[:], consts_d[:]).then_inc(sem["sca"], 16)
                # scatter stat rows into [16,128] once rows are complete
                sync.wait_ge(sem["sv"], 8)
                sync.dma_start(sum16[:], sumrow[:]).then_inc(sem["sscat"], 16)
                sync.wait_ge(sem["sa"], 7)
                sync.wait_ge(sem["sv"], 9)
                sync.dma_start(q16s[:], qrow[:]).then_inc(sem["qscat"], 16)
                # output (completion waited on gpsimd, overlapping exit barrier)
                sync.wait_ge(sem["sv"], 47)
                sync.dma_start(loss_d[:], loss16[:]).then_inc(sem["dout"], 16)

            @block.gpsimd
            def _(gp):
                gp.wait_ge(sem["dz2"], 16)
                gp.tensor_copy(zb[:, blksl[2]], z[:, blksl[2]]).then_inc(sem["sg"])       # 1
                gp.wait_ge(sem["dz3"], 16)
                gp.tensor_copy(zb[:, blksl[3]], z[:, blksl[3]]).then_inc(sem["sg"])       # 2
                gp.wait_ge(sem["dout"], 16)

            @block.scalar
            def _(act):
                act.wait_ge(sem["dz0"], 16)
                act.square(sqb[:, blksl[0]], z[:, blksl[0]]).then_inc(sem["sa"])          # 1
                act.wait_ge(sem["dz1"], 16)
                act.square(sqb[:, blksl[1]], z[:, blksl[1]]).then_inc(sem["sa"])          # 2
                act.wait_ge(sem["dz2"], 16)
                act.square(sqb[:, blksl[2]], z[:, blksl[2]]).then_inc(sem["sa"])          # 3
                act.wait_ge(sem["dz3"], 16)
                act.square(sqb[:, blksl[3]], z[:, blksl[3]]).then_inc(sem["sa"])          # 4
                act.wait_ge(sem["spe"], 2)
                act.copy(qrow[:, blksl[0]], ps_q_t[0][:]).then_inc(sem["sa"])             # 5
                act.wait_ge(sem["spe"], 4)
                act.copy(qrow[:, blksl[1]], ps_q_t[1][:]).then_inc(sem["sa"])             # 6
                act.wait_ge(sem["spe"], 8)
                act.copy(qrow[:, blksl[3]], ps_q_t[3][:]).then_inc(sem["sa"])             # 7
                act.wait_ge(sem["sv"], 11)
                act.activation(std16[:], var16[:], AF.Sqrt).then_inc(sem["sa"])           # 8
                act.wait_ge(sem["sv"], 24)
                act.activation(u[:], z2n[:], AF.Square, bias=negone[:],
                               scale=a_sb[:]).then_inc(sem["sa"])                         # 9
                act.wait_ge(sem["spe"], 14)
                act.copy(q16sb[:], q16[:]).then_inc(sem["sa"])                            # 10

            @block.vector
            def _(dve):
                dve.memset(negone[:], -1.0).then_inc(sem["sv"])                           # 1
                dve.memset(ones_b[:], 1.0).then_inc(sem["sv"])                            # 2
                dve.wait_ge(sem["dz0"], 16)
                dve.tensor_copy(zb[:, blksl[0]], z[:, blksl[0]]).then_inc(sem["sv"])      # 3
                dve.wait_ge(sem["dz1"], 16)
                dve.tensor_copy(zb[:, blksl[1]], z[:, blksl[1]]).then_inc(sem["sv"])      # 4
                for i in range(4):                                                        # 5-8
                    dve.wait_ge(sem["spe"], 2 * i + 1)
                    dve.tensor_copy(sumrow[:, blksl[i]], ps_s_t[i][:]).then_inc(sem["sv"])
                dve.wait_ge(sem["spe"], 6)
                dve.tensor_copy(qrow[:, blksl[2]], ps_q_t[2][:]).then_inc(sem["sv"])      # 9
                dve.wait_ge(sem["sscat"], 16)
                dve.scalar_tensor_tensor(
                    t1[:], sum16[:], 1.0 / (B * (B - 1.0)), sum16[:],
                    op0=AL.mult, op1=AL.mult).then_inc(sem["sv"])                         # 10
                dve.wait_ge(sem["qscat"], 16)
                dve.scalar_tensor_tensor(
                    var16[:], q16s[:], 1.0 / (B - 1.0), t1[:],
                    op0=AL.mult, op1=AL.subtract).then_inc(sem["sv"])                     # 11
                dve.wait_ge(sem["sa"], 8)
                dve.reciprocal(A_z[:], std16[:]).then_inc(sem["sv"])                      # 12
                dve.scalar_tensor_tensor(
                    C_z[:], sum16[:], 1.0 / B, A_z[:],
                    op0=AL.mult, op1=AL.mult).then_inc(sem["sv"])                         # 13
                # normalize.  Wait for ALL four broadcast matmuls first:
                # concurrent PE-write + DVE-read of one PSUM bank is fatal.
                dve.wait_ge(sem["sca"], 16)
                dve.wait_ge(sem["spe"], 12)
                dve.tensor_tensor(tn1[:], z1r, psA1, AL.mult).then_inc(sem["sv"])         # 14
                dve.tensor_tensor(z1n[:], tn1[:], psC1, AL.subtract).then_inc(sem["sv"])  # 15
                dve.tensor_tensor(tn2[:], z2r, psA2, AL.mult).then_inc(sem["sv"])         # 16
                dve.tensor_tensor(z2n[:], tn2[:], psC2, AL.subtract).then_inc(sem["sv"])  # 17
                dve.scalar_tensor_tensor(
                    w[:], z1n[:], 1.0, z2n[:], op0=AL.bypass, op1=AL.mult,
                    accum_out=colsD[:, 0:1]).then_inc(sem["sv"])                          # 18 R
                dve.scalar_tensor_tensor(
                    junkP[:], w[:], 1.0, w[:], op0=AL.bypass, op1=AL.mult,
                    accum_out=colsD[:, 1:2]).then_inc(sem["sv"])                          # 19 P
                dve.scalar_tensor_tensor(
                    junkA2[:], z1n[:], 1.0, z1n[:], op0=AL.bypass, op1=AL.mult,
                    accum_out=colsD[:, 6:7]).then_inc(sem["sv"])                          # 20 Sa
                dve.scalar_tensor_tensor(
                    junkV2[:], z2n[:], 1.0, z2n[:], op0=AL.bypass, op1=AL.mult,
                    accum_out=colsD[:, 7:8]).then_inc(sem["sv"])                          # 21 Sv
                dve.scalar_tensor_tensor(
                    junkA[:], z1n[:], 1.0, amask, op0=AL.bypass, op1=AL.mult,
                    accum_out=acol[:]).then_inc(sem["sv"])                                # 22
                dve.scalar_tensor_tensor(
                    junkV[:], z2n[:], 1.0, amask, op0=AL.bypass, op1=AL.mult,
                    accum_out=vcol[:]).then_inc(sem["sv"])                                # 23
                dve.wait_ge(sem["spe"], 13)
                dve.tensor_copy(a_sb[:], a_ps[:]).then_inc(sem["sv"])                     # 24
                dve.wait_ge(sem["sa"], 9)
                dve.scalar_tensor_tensor(
                    junkQ[:], u[:], 1.0, u[:], op0=AL.bypass, op1=AL.mult,
                    accum_out=colsD[:, 2:3]).then_inc(sem["sv"])                          # 25 Q
                dve.scalar_tensor_tensor(
                    junkG[:], u[:], 1.0, amask, op0=AL.bypass, op1=AL.mult,
                    accum_out=colsD[:, 3:4]).then_inc(sem["sv"])                          # 26 gd
                dve.tensor_tensor(colsD[:, 4:5], acol[:], vcol[:],
                                  AL.mult).then_inc(sem["sv"])                            # 27 d
                dve.tensor_tensor(colsD[:, 5:6], acol[:], acol[:],
                                  AL.mult).then_inc(sem["sv"])                            # 28 a2
                # ---- finals on [16,1]; three interleaved chains keep >=2-op
                #      spacing for same-engine RAW on tiny tensors ----
                R_ = q16sb[:, 0:1]
                P_ = q16sb[:, 1:2]
                Q_ = q16sb[:, 2:3]
                gd_ = q16sb[:, 3:4]
                d_ = q16sb[:, 4:5]
                a2_ = q16sb[:, 5:6]
                Sa_ = q16sb[:, 6:7]
                Sv_ = q16sb[:, 7:8]
                d2 = fin[:, 0:1]
                g4 = fin[:, 1:2]
                h = fin[:, 2:3]
                Tp = fin[:, 3:4]
                on1 = fin[:, 4:5]
                on2 = fin[:, 5:6]
                e1 = fin[:, 6:7]
                f1 = fin[:, 7:8]
                f2 = fin[:, 8:9]
                u1 = fin[:, 9:10]
                u2 = fin[:, 10:11]
                off = fin[:, 11:12]
                hm = fin[:, 12:13]
                Tp2 = fin[:, 13:14]
                dve.wait_ge(sem["sa"], 10)
                dve.tensor_tensor(d2, d_, d_, AL.mult).then_inc(sem["sv"])       # 29
                dve.tensor_tensor(g4, gd_, gd_, AL.mult).then_inc(sem["sv"])     # 30
                dve.tensor_scalar_add(hm, gd_, -1.0).then_inc(sem["sv"])         # 31
                dve.scalar_tensor_tensor(
                    Tp, R_, -2.0, P_, op0=AL.mult, op1=AL.add).then_inc(sem["sv"])  # 32
                dve.tensor_tensor(e1, Sa_, a2_, AL.subtract).then_inc(sem["sv"])    # 33
                dve.tensor_tensor(h, hm, hm, AL.mult).then_inc(sem["sv"])        # 34 [hm+2]
                dve.tensor_tensor(f2, d2, Q_, AL.add).then_inc(sem["sv"])        # 35 [d2+4]
                dve.tensor_scalar_add(Tp2, Tp, float(D)).then_inc(sem["sv"])     # 36 [Tp+3]
                dve.tensor_tensor(f1, e1, Sv_, AL.mult).then_inc(sem["sv"])      # 37 [e1+3]
                dve.tensor_tensor(u2, f2, P_, AL.subtract).then_inc(sem["sv"])   # 38 [f2+2]
                dve.scalar_tensor_tensor(
                    on1, gd_, -1.0, Tp2,
                    op0=AL.mult, op1=AL.add).then_inc(sem["sv"])                 # 39 [Tp2+2]
                dve.tensor_tensor(u1, f1, g4, AL.subtract).then_inc(sem["sv"])   # 40 [f1+2]
                dve.drain().then_inc(sem["sv"])                                  # 41
                dve.tensor_tensor(on2, on1, h, AL.add).then_inc(sem["sv"])       # 42
                dve.tensor_tensor(off, u1, u2, AL.add).then_inc(sem["sv"])       # 43
                dve.drain().then_inc(sem["sv"])                                  # 44
                dve.scalar_tensor_tensor(
                    loss16[:], off, LAM, on2,
                    op0=AL.mult, op1=AL.add).then_inc(sem["sv"])                 # 45
                dve.drain().then_inc(sem["sv"])                                  # 46
                dve.engine_nop().then_inc(sem["sv"])                             # 47

            @block.tensor
            def _(pe):
                pe.wait_ge(sem["sv"], 3)
                pe.matmul(ps_s_t[0][:], ones_b[:], zb[:, blksl[0]],
                          start=True, stop=True).then_inc(sem["spe"])                     # 1
                pe.wait_ge(sem["sa"], 1)
                pe.matmul(ps_q_t[0][:], ones_b[:], sqb[:, blksl[0]],
                          start=True, stop=True).then_inc(sem["spe"])                     # 2
                pe.wait_ge(sem["sv"], 4)
                pe.matmul(ps_s_t[1][:], ones_b[:], zb[:, blksl[1]],
                          start=True, stop=True).then_inc(sem["spe"])                     # 3
                pe.wait_ge(sem["sa"], 2)
                pe.matmul(ps_q_t[1][:], ones_b[:], sqb[:, blksl[1]],
                          start=True, stop=True).then_inc(sem["spe"])                     # 4
                pe.wait_ge(sem["sg"], 1)
                pe.wait_ge(sem["sv"], 5)
                pe.matmul(ps_s_t[2][:], ones_b[:], zb[:, blksl[2]], start=True,
                          stop=True, skip_group_check=True).then_inc(sem["spe"])          # 5
                pe.wait_ge(sem["sa"], 5)
                pe.matmul(ps_q_t[2][:], ones_b[:], sqb[:, blksl[2]], start=True,
                          stop=True, skip_group_check=True).then_inc(sem["spe"])          # 6
                pe.wait_ge(sem["sg"], 2)
                pe.wait_ge(sem["sv"], 6)
                pe.matmul(ps_s_t[3][:], ones_b[:], zb[:, blksl[3]], start=True,
                          stop=True, skip_group_check=True).then_inc(sem["spe"])          # 7
                pe.wait_ge(sem["sa"], 6)
                pe.matmul(ps_q_t[3][:], ones_b[:], sqb[:, blksl[3]], start=True,
                          stop=True, skip_group_check=True).then_inc(sem["spe"])          # 8
                # broadcasts: 9-12
                pe.wait_ge(sem["sca"], 16)
                pe.wait_ge(sem["sv"], 12)
                pe.matmul(psA1, selz1, A_z[:], start=True,
                          stop=True).then_inc(sem["spe"])                                 # 9
                pe.wait_ge(sem["sv"], 13)
                pe.matmul(psC1, selz1, C_z[:], start=True, stop=True,
                          skip_group_check=True).then_inc(sem["spe"])                     # 10
                pe.matmul(psA2, selz2, A_z[:], start=True, stop=True,
                          skip_group_check=True).then_inc(sem["spe"])                     # 11
                pe.matmul(psC2, selz2, C_z[:], start=True, stop=True,
                          skip_group_check=True).then_inc(sem["spe"])                     # 12
                # expand a: 13
                pe.wait_ge(sem["sv"], 22)
                pe.matmul(a_ps[:], expand, acol[:], start=True,
                          stop=True).then_inc(sem["spe"])                                 # 13
                # group reduce: 14
                pe.wait_ge(sem["sv"], 28)
                pe.matmul(q16[:], gsel, colsD[:], start=True,
                          stop=True).then_inc(sem["spe"])                                 # 14

    return nc


def _host_inputs(z1, z2):
    """Per-core input maps (sharding glue)."""
    z1 = np.ascontiguousarray(z1, np.float32)
    z2 = np.ascontiguousarray(z2, np.float32)

    base = np.zeros((128, C_TOTAL), np.float32)
    for m in range(128):
        base[8 * (m // 8), C_EXPAND + m] = 1.0   # expand
        base[m, C_GSEL + m // 8] = 1.0           # gsel
        base[m % 8, C_SELZ1 + m] = 1.0           # selz1
        base[8 + m % 8, C_SELZ2 + m] = 1.0       # selz2

    in_maps = []
    for c in range(NCORES):
        rows = slice(c * SPC, (c + 1) * SPC)
        consts = base.copy()
        consts[:, C_Z1R:C_Z1R + 128] = \
            z1[rows].reshape(SPC, KCH, 128).reshape(128, 128)
        consts[:, C_Z2R:C_Z2R + 128] = \
            z2[rows].reshape(SPC, KCH, 128).reshape(128, 128)
        for s in range(SPC):
            consts[s * 8, C_AMASK + c * SPC + s] = 1.0
        in_maps.append({
            "z1": z1, "z2": z2,
            "consts": np.ascontiguousarray(consts),
        })
    return in_maps


_cached_nc = None


def run(z1, z2, trace=False, **kwargs):
    global _cached_nc
    if _cached_nc is None:
        _cached_nc = build_program()
    in_maps = _host_inputs(z1, z2)
    res = run_bass_kernel_spmd(
        _cached_nc, in_maps, core_ids=list(range(NCORES)), trace=trace, **kwargs)
    out = np.concatenate([res.results[c]["loss"][:, 0] for c in range(NCORES)])
    return out.astype(np.float32), res


def kernel(z1, z2):
    out, _ = run(z1, z2, trace=False)
    return out


# revision 38
# speedup vs baseline: 1.2217x; 1.0716x over previous
"""BarlowTwins-style loss kernel for Trainium2 (raw Bass), 8-core SPMD.

Math: the reference materializes a (B, D, D) per-sample cross-correlation
tensor, but the loss algebraically reduces to O(B*D) work.  With
z1n/z2n the per-dim (batch-)normalized inputs and per-sample b:
    w    = z1n[b,:] * z2n[b,:]
    R    = sum(w);  P = sum(w^2);  Sa = sum(z1n^2);  Sv = sum(z2n^2)
    a    = z1n[b,b];  v = z2n[b,b];  d = a*v;  g2 = (d-1)^2
    u    = (a*z2n[b,:] - 1)^2;  Q = sum(u^2);  (g2 == u[b])
    T    = P - 2R + D                  # sum((w-1)^2)
    on   = T - g2 + (g2-1)^2
    off  = (Sa - a^2)*Sv - P + d^2 + Q - g2^2
    loss = on + 0.005*off

Sharding: data-parallel over batch.  Every core loads the full z1/z2 to
compute per-dim column sums/sumsq locally (cheaper than an all-reduce at
this size), then computes the loss for its own 16 samples using a
rearranged layout [128 partitions = (sample, col-chunk), 128 free] so all
row-reductions run at full partition occupancy.

Written in raw Bass (explicit semaphores): the walrus build in this
container only supports a single sync-wait per instruction, which the
Tile framework's exit sequence violates; standalone wait_ge instructions
compose freely.
"""

import sys
from contextlib import ExitStack

import numpy as np

for _p in ("/opt/trn_rl_repo",):
    if _p not in sys.path:
        sys.path.append(_p)

import concourse.bass as bass
import concourse.mybir as mybir
from concourse.bass_utils import run_bass_kernel_spmd

B, D = 128, 1024
NCORES = 8
SPC = B // NCORES  # 16 samples per core
KCH = D // 128     # 8 column chunks of 128
LAM = 0.005

FP = mybir.dt.float32
BF = mybir.dt.bfloat16
AF = mybir.ActivationFunctionType
AL = mybir.AluOpType

# consts column layout
C_AMASK = 0
C_EXPAND = 128
C_Z1R = 256
C_Z2R = 384
C_GSEL = 512
C_SELZ1 = 528
C_SELZ2 = 656
C_TOTAL = 784


def build_program():
    nc = bass.Bass("TRN2", debug=False, num_devices=NCORES,
                   detect_race_conditions=False)

    z1_d = nc.dram_tensor("z1", [B, D], FP, kind="ExternalInput")
    z2_d = nc.dram_tensor("z2", [B, D], FP, kind="ExternalInput")
    consts_d = nc.dram_tensor("consts", [128, C_TOTAL], FP, kind="ExternalInput")
    loss_d = nc.dram_tensor("loss", [SPC, 1], FP, kind="ExternalOutput")

    ctx = ExitStack()
    with ctx:
        sem = {n: ctx.enter_context(nc.semaphore(n)) for n in
               ["sca", "dz0", "dz1", "dz2", "dz3", "sscat", "qscat", "dout",
                "spe", "sv", "sa", "sg"]}

        def sb(name, shape, dtype=FP):
            return ctx.enter_context(nc.sbuf_tensor(name, shape, dtype))

        ca = sb("ca", [128, C_TOTAL])
        z = sb("z", [128, 2 * D])
        zb = sb("zb", [128, 2 * D], BF)
        sqb = sb("sqb", [128, 2 * D], BF)
        sumrow = sb("sumrow", [1, 2 * D])
        qrow = sb("qrow", [1, 2 * D])
        sum16 = sb("sum16", [16, 128])
        q16s = sb("q16s", [16, 128])
        q127 = sb("q127", [16, 128])
        t1 = sb("t1", [16, 128])
        var16 = sb("var16", [16, 128])
        std16 = sb("std16", [16, 128])
        A_z = sb("A_z", [16, 128])
        C_z = sb("C_z", [16, 128])
        tn1 = sb("tn1", [128, 128])
        z1n = sb("z1n", [128, 128])
        tn2 = sb("tn2", [128, 128])
        z2n = sb("z2n", [128, 128])
        w = sb("w", [128, 128])
        u = sb("u", [128, 128])
        junkP = sb("junkP", [128, 128])
        junkQ = sb("junkQ", [128, 128])
        junkA = sb("junkA", [128, 128])
        junkV = sb("junkV", [128, 128])
        junkG = sb("junkG", [128, 128])
        junkA2 = sb("junkA2", [128, 128])
        junkV2 = sb("junkV2", [128, 128])
        negone = sb("negone", [128, 1])
        ones_b = sb("ones_b", [128, 1], BF)
        acol = sb("acol", [128, 1])
        vcol = sb("vcol", [128, 1])
        a_sb = sb("a_sb", [128, 1])
        colsD = sb("colsD", [128, 8])
        q16sb = sb("q16sb", [16, 8])
        fin = sb("fin", [16, 14])
        loss16 = sb("loss16", [16, 1])

        # PSUM: 4 stat banks (reused by blocks 2,3) + broadcast bank +
        # a-expand bank + group-reduce bank = 7 of 8 banks.
        ps = [ctx.enter_context(nc.psum_tensor(f"ps{i}", [1, 512], FP))
              for i in range(4)]
        ps_s_t = [ps[0], ps[2], ps[0], ps[2]]
        ps_q_t = [ps[1], ps[3], ps[1], ps[3]]
        psBC1 = ctx.enter_context(nc.psum_tensor("psBC1", [128, 256], FP))
        psBC2 = ctx.enter_context(nc.psum_tensor("psBC2", [128, 256], FP))
        q16 = ctx.enter_context(nc.psum_tensor("q16", [16, 8], FP))

        psA1 = psBC1[:, 0:128]
        psC1 = psBC1[:, 128:256]
        psA2 = psBC2[:, 0:128]
        psC2 = psBC2[:, 128:256]

        amask = ca[:, C_AMASK:C_AMASK + 128]
        expand = ca[:, C_EXPAND:C_EXPAND + 128]
        z1r = ca[:, C_Z1R:C_Z1R + 128]
        z2r = ca[:, C_Z2R:C_Z2R + 128]
        gsel = ca[:, C_GSEL:C_GSEL + 16]
        selz1 = ca[0:16, C_SELZ1:C_SELZ1 + 128]
        selz2 = ca[0:16, C_SELZ2:C_SELZ2 + 128]

        blksl = [slice(i * 512, (i + 1) * 512) for i in range(4)]

        with nc.Block() as block:

            @block.sync
            def _(sync):
                sync.dma_start(z[:, blksl[0]], z1_d[:, 0:512]).then_inc(sem["dz0"], 16)
                sync.dma_start(z[:, blksl[1]], z1_d[:, 512:1024]).then_inc(sem["dz1"], 16)
                sync.dma_start(z[:, blksl[2]], z2_d[:, 0:512]).then_inc(sem["dz2"], 16)
                sync.dma_start(z[:, blksl[3]], z2_d[:, 512:1024]).then_inc(sem["dz3"], 16)
                sync.dma_start(ca[:], consts_d[:]).then_inc(sem["sca"], 16)
                sync.wait_ge(sem["sv"], 10)
                sync.dma_start(sum16[:], sumrow[:]).then_inc(sem["sscat"], 16)
                sync.wait_ge(sem["sa"], 8)
                sync.dma_start(q16s[:], qrow[:]).then_inc(sem["qscat"], 16)
                sync.wait_ge(sem["sv"], 48)
                sync.dma_start(loss_d[:], loss16[:]).then_inc(sem["dout"], 16)

            @block.gpsimd
            def _(gp):
                gp.wait_ge(sem["dout"], 16)

            @block.scalar
            def _(act):
                act.wait_ge(sem["dz0"], 16)
                act.square(sqb[:, blksl[0]], z[:, blksl[0]]).then_inc(sem["sa"])          # 1
                act.wait_ge(sem["dz1"], 16)
                act.square(sqb[:, blksl[1]], z[:, blksl[1]]).then_inc(sem["sa"])          # 2
                act.wait_ge(sem["dz2"], 16)
                act.square(sqb[:, blksl[2]], z[:, blksl[2]]).then_inc(sem["sa"])          # 3
                act.wait_ge(sem["dz3"], 16)
                act.square(sqb[:, blksl[3]], z[:, blksl[3]]).then_inc(sem["sa"])          # 4
                for i in range(4):                                                        # 5-8
                    act.wait_ge(sem["spe"], 2 * i + 2)
                    act.copy(qrow[:, blksl[i]], ps_q_t[i][:]).then_inc(sem["sa"])
                act.wait_ge(sem["sv"], 12)
                act.activation(std16[:], var16[:], AF.Sqrt).then_inc(sem["sa"])           # 9
                act.wait_ge(sem["sv"], 25)
                act.activation(u[:], z2n[:], AF.Square, bias=negone[:],
                               scale=a_sb[:]).then_inc(sem["sa"])                         # 10
                act.wait_ge(sem["spe"], 13)
                act.copy(q16sb[:], q16[:]).then_inc(sem["sa"])                            # 11

            @block.vector
            def _(dve):
                dve.memset(negone[:], -1.0).then_inc(sem["sv"])                           # 1
                dve.memset(ones_b[:], 1.0).then_inc(sem["sv"])                            # 2
                dve.wait_ge(sem["dz0"], 16)
                dve.tensor_copy(zb[:, blksl[0]], z[:, blksl[0]]).then_inc(sem["sv"])      # 3
                dve.wait_ge(sem["dz1"], 16)
                dve.tensor_copy(zb[:, blksl[1]], z[:, blksl[1]]).then_inc(sem["sv"])      # 4
                dve.wait_ge(sem["dz2"], 16)
                dve.tensor_copy(zb[:, blksl[2]], z[:, blksl[2]]).then_inc(sem["sv"])      # 5
                dve.wait_ge(sem["dz3"], 16)
                dve.tensor_copy(zb[:, blksl[3]], z[:, blksl[3]]).then_inc(sem["sv"])      # 6
                for i in range(4):                                                        # 7-10
                    dve.wait_ge(sem["spe"], 2 * i + 1)
                    dve.tensor_copy(sumrow[:, blksl[i]], ps_s_t[i][:]).then_inc(sem["sv"])
                dve.wait_ge(sem["sscat"], 16)
                dve.scalar_tensor_tensor(
                    t1[:], sum16[:], 1.0 / (B * (B - 1.0)), sum16[:],
                    op0=AL.mult, op1=AL.mult).then_inc(sem["sv"])                         # 11
                dve.wait_ge(sem["qscat"], 16)
                dve.scalar_tensor_tensor(
                    var16[:], q16s[:], 1.0 / (B - 1.0), t1[:],
                    op0=AL.mult, op1=AL.subtract).then_inc(sem["sv"])                     # 12
                dve.wait_ge(sem["sa"], 9)
                dve.reciprocal(A_z[:], std16[:]).then_inc(sem["sv"])                      # 13
                dve.scalar_tensor_tensor(
                    C_z[:], sum16[:], 1.0 / B, A_z[:],
                    op0=AL.mult, op1=AL.mult).then_inc(sem["sv"])                         # 14
                # normalize z1 after its bank's two matmuls; z2 after the rest
                dve.wait_ge(sem["sca"], 16)
                dve.wait_ge(sem["spe"], 10)
                dve.tensor_tensor(tn1[:], z1r, psA1, AL.mult).then_inc(sem["sv"])         # 15
                dve.tensor_tensor(z1n[:], tn1[:], psC1, AL.subtract).then_inc(sem["sv"])  # 16
                dve.wait_ge(sem["spe"], 12)
                dve.tensor_tensor(tn2[:], z2r, psA2, AL.mult).then_inc(sem["sv"])         # 17
                dve.tensor_tensor(z2n[:], tn2[:], psC2, AL.subtract).then_inc(sem["sv"])  # 18
                dve.scalar_tensor_tensor(
                    w[:], z1n[:], 1.0, z2n[:], op0=AL.bypass, op1=AL.mult,
                    accum_out=colsD[:, 0:1]).then_inc(sem["sv"])                          # 19 R
                dve.scalar_tensor_tensor(
                    junkP[:], w[:], 1.0, w[:], op0=AL.bypass, op1=AL.mult,
                    accum_out=colsD[:, 1:2]).then_inc(sem["sv"])                          # 20 P
                dve.scalar_tensor_tensor(
                    junkA[:], z1n[:], 1.0, amask, op0=AL.bypass, op1=AL.mult,
                    accum_out=acol[:]).then_inc(sem["sv"])                                # 21
                dve.scalar_tensor_tensor(
                    junkV[:], z2n[:], 1.0, amask, op0=AL.bypass, op1=AL.mult,
                    accum_out=vcol[:]).then_inc(sem["sv"])                                # 22
                dve.scalar_tensor_tensor(
                    junkA2[:], z1n[:], 1.0, z1n[:], op0=AL.bypass, op1=AL.mult,
                    accum_out=colsD[:, 6:7]).then_inc(sem["sv"])                          # 23 Sa
                dve.scalar_tensor_tensor(
                    junkV2[:], z2n[:], 1.0, z2n[:], op0=AL.bypass, op1=AL.mult,
                    accum_out=colsD[:, 7:8]).then_inc(sem["sv"])                          # 24 Sv
                # a_r: broadcast acol within each 8-partition group (acol gap 3)
                dve.stream_shuffle(a_sb[:], acol[:],
                                   [8 * (i // 8) for i in range(32)]).then_inc(sem["sv"])  # 25
                dve.wait_ge(sem["sa"], 10)
                dve.scalar_tensor_tensor(
                    junkQ[:], u[:], 1.0, u[:], op0=AL.bypass, op1=AL.mult,
                    accum_out=colsD[:, 2:3]).then_inc(sem["sv"])                          # 26 Q
                dve.scalar_tensor_tensor(
                    junkG[:], u[:], 1.0, amask, op0=AL.bypass, op1=AL.mult,
                    accum_out=colsD[:, 3:4]).then_inc(sem["sv"])                          # 27 gd
                dve.tensor_tensor(colsD[:, 4:5], acol[:], vcol[:],
                                  AL.mult).then_inc(sem["sv"])                            # 28 d
                dve.tensor_tensor(colsD[:, 5:6], acol[:], acol[:],
                                  AL.mult).then_inc(sem["sv"])                            # 29 a2
                # ---- finals on [16,1]; interleaved chains keep >=2-op spacing ----
                R_ = q16sb[:, 0:1]
                P_ = q16sb[:, 1:2]
                Q_ = q16sb[:, 2:3]
                gd_ = q16sb[:, 3:4]
                d_ = q16sb[:, 4:5]
                a2_ = q16sb[:, 5:6]
                Sa_ = q16sb[:, 6:7]
                Sv_ = q16sb[:, 7:8]
                d2 = fin[:, 0:1]
                g4 = fin[:, 1:2]
                h = fin[:, 2:3]
                Tp = fin[:, 3:4]
                on1 = fin[:, 4:5]
                on2 = fin[:, 5:6]
                e1 = fin[:, 6:7]
                f1 = fin[:, 7:8]
                f2 = fin[:, 8:9]
                u1 = fin[:, 9:10]
                u2 = fin[:, 10:11]
                off = fin[:, 11:12]
                hm = fin[:, 12:13]
                Tp2 = fin[:, 13:14]
                dve.wait_ge(sem["sa"], 11)
                dve.tensor_tensor(d2, d_, d_, AL.mult).then_inc(sem["sv"])       # 30
                dve.tensor_tensor(g4, gd_, gd_, AL.mult).then_inc(sem["sv"])     # 31
                dve.tensor_scalar_add(hm, gd_, -1.0).then_inc(sem["sv"])         # 32
                dve.scalar_tensor_tensor(
                    Tp, R_, -2.0, P_, op0=AL.mult, op1=AL.add).then_inc(sem["sv"])  # 33
                dve.tensor_tensor(e1, Sa_, a2_, AL.subtract).then_inc(sem["sv"])    # 34
                dve.tensor_tensor(h, hm, hm, AL.mult).then_inc(sem["sv"])        # 35
                dve.tensor_tensor(f2, d2, Q_, AL.add).then_inc(sem["sv"])        # 36
                dve.tensor_scalar_add(Tp2, Tp, float(D)).then_inc(sem["sv"])     # 37
                dve.tensor_tensor(f1, e1, Sv_, AL.mult).then_inc(sem["sv"])      # 38
                dve.tensor_tensor(u2, f2, P_, AL.subtract).then_inc(sem["sv"])   # 39
                dve.scalar_tensor_tensor(
                    on1, gd_, -1.0, Tp2,
                    op0=AL.mult, op1=AL.add).then_inc(sem["sv"])                 # 40
                dve.tensor_tensor(u1, f1, g4, AL.subtract).then_inc(sem["sv"])   # 41
                dve.drain().then_inc(sem["sv"])                                  # 42
                dve.tensor_tensor(on2, on1, h, AL.add).then_inc(sem["sv"])       # 43
                dve.tensor_tensor(off, u1, u2, AL.add).then_inc(sem["sv"])       # 44
                dve.drain().then_inc(sem["sv"])                                  # 45
                dve.scalar_tensor_tensor(
                    loss16[:], off, LAM, on2,
                    op0=AL.mult, op1=AL.add).then_inc(sem["sv"])                 # 46
                dve.drain().then_inc(sem["sv"])                                  # 47
                dve.engine_nop().then_inc(sem["sv"])                             # 48

            @block.tensor
            def _(pe):
                pe.wait_ge(sem["sv"], 3)
                pe.matmul(ps_s_t[0][:], ones_b[:], zb[:, blksl[0]],
                          start=True, stop=True).then_inc(sem["spe"])                     # 1
                pe.wait_ge(sem["sa"], 1)
                pe.matmul(ps_q_t[0][:], ones_b[:], sqb[:, blksl[0]],
                          start=True, stop=True).then_inc(sem["spe"])                     # 2
                pe.wait_ge(sem["sv"], 4)
                pe.matmul(ps_s_t[1][:], ones_b[:], zb[:, blksl[1]],
                          start=True, stop=True).then_inc(sem["spe"])                     # 3
                pe.wait_ge(sem["sa"], 2)
                pe.matmul(ps_q_t[1][:], ones_b[:], sqb[:, blksl[1]],
                          start=True, stop=True).then_inc(sem["spe"])                     # 4
                pe.wait_ge(sem["sv"], 7)   # zb2 + WAR s0-row copy
                pe.matmul(ps_s_t[2][:], ones_b[:], zb[:, blksl[2]], start=True,
                          stop=True, skip_group_check=True).then_inc(sem["spe"])          # 5
                pe.wait_ge(sem["sa"], 5)   # sq2 + WAR q0-row copy
                pe.matmul(ps_q_t[2][:], ones_b[:], sqb[:, blksl[2]], start=True,
                          stop=True, skip_group_check=True).then_inc(sem["spe"])          # 6
                pe.wait_ge(sem["sv"], 8)   # zb3 + WAR s1-row copy
                pe.matmul(ps_s_t[3][:], ones_b[:], zb[:, blksl[3]], start=True,
                          stop=True, skip_group_check=True).then_inc(sem["spe"])          # 7
                pe.wait_ge(sem["sa"], 6)   # sq3 + WAR q1-row copy
                pe.matmul(ps_q_t[3][:], ones_b[:], sqb[:, blksl[3]], start=True,
                          stop=True, skip_group_check=True).then_inc(sem["spe"])          # 8
                # broadcasts: bank A (A1,C1) then bank B (A2,C2)
                pe.wait_ge(sem["sca"], 16)
                pe.wait_ge(sem["sv"], 13)
                pe.matmul(psA1, selz1, A_z[:], start=True,
                          stop=True).then_inc(sem["spe"])                                 # 9
                pe.wait_ge(sem["sv"], 14)
                pe.matmul(psC1, selz1, C_z[:], start=True, stop=True,
                          skip_group_check=True).then_inc(sem["spe"])                     # 10
                pe.matmul(psA2, selz2, A_z[:], start=True, stop=True,
                          skip_group_check=True).then_inc(sem["spe"])                     # 11
                pe.matmul(psC2, selz2, C_z[:], start=True, stop=True,
                          skip_group_check=True).then_inc(sem["spe"])                     # 12
                # group reduce
                pe.wait_ge(sem["sv"], 29)
                pe.matmul(q16[:], gsel, colsD[:], start=True,
                          stop=True).then_inc(sem["spe"])                                 # 13

    return nc


def _host_inputs(z1, z2):
    """Per-core input maps (sharding glue)."""
    z1 = np.ascontiguousarray(z1, np.float32)
    z2 = np.ascontiguousarray(z2, np.float32)

    base = np.zeros((128, C_TOTAL), np.float32)
    for m in range(128):
        base[8 * (m // 8), C_EXPAND + m] = 1.0   # expand
        base[m, C_GSEL + m // 8] = 1.0           # gsel
        base[m % 8, C_SELZ1 + m] = 1.0           # selz1
        base[8 + m % 8, C_SELZ2 + m] = 1.0       # selz2

    in_maps = []
    for c in range(NCORES):
        rows = slice(c * SPC, (c + 1) * SPC)
        consts = base.copy()
        consts[:, C_Z1R:C_Z1R + 128] = \
            z1[rows].reshape(SPC, KCH, 128).reshape(128, 128)
        consts[:, C_Z2R:C_Z2R + 128] = \
            z2[rows].reshape(SPC, KCH, 128).reshape(128, 128)
        for s in range(SPC):
            consts[s * 8, C_AMASK + c * SPC + s] = 1.0
        in_maps.append({
            "z1": z1, "z2": z2,
            "consts": np.ascontiguousarray(consts),
        })
    return in_maps


_cached_nc = None


def run(z1, z2, trace=False, **kwargs):
    global _cached_nc
    if _cached_nc is None:
        _cached_nc = build_program()
    in_maps = _host_inputs(z1, z2)
    res = run_bass_kernel_spmd(
        _cached_nc, in_maps, core_ids=list(range(NCORES)), trace=trace, **kwargs)
    out = np.concatenate([res.results[c]["loss"][:, 0] for c in range(NCORES)])
    return out.astype(np.float32), res


def kernel(z1, z2):
    out, _ = run(z1, z2, trace=False)
    return out


# revision 40
# speedup vs baseline: 1.2340x; 1.0101x over previous
"""BarlowTwins-style loss kernel for Trainium2 (raw Bass), 8-core SPMD.

Math: the reference materializes a (B, D, D) per-sample cross-correlation
tensor, but the loss algebraically reduces to O(B*D) work.  With
z1n/z2n the per-dim (batch-)normalized inputs and per-sample b:
    w    = z1n[b,:] * z2n[b,:]
    R    = sum(w);  P = sum(w^2);  Sa = sum(z1n^2);  Sv = sum(z2n^2)
    a    = z1n[b,b];  v = z2n[b,b];  d = a*v;  g2 = (d-1)^2
    u    = (a*z2n[b,:] - 1)^2;  Q = sum(u^2);  (g2 == u[b])
    T    = P - 2R + D                  # sum((w-1)^2)
    on   = T - g2 + (g2-1)^2
    off  = (Sa - a^2)*Sv - P + d^2 + Q - g2^2
    loss = on + 0.005*off

Sharding: data-parallel over batch.  Every core loads the full z1/z2 to
compute per-dim column sums/sumsq locally (cheaper than an all-reduce at
this size), then computes the loss for its own 16 samples using a
rearranged layout [128 partitions = (sample, col-chunk), 128 free] so all
row-reductions run at full partition occupancy.

Written in raw Bass (explicit semaphores): the walrus build in this
container only supports a single sync-wait per instruction, which the
Tile framework's exit sequence violates; standalone wait_ge instructions
compose freely.
"""

import sys
from contextlib import ExitStack

import numpy as np

for _p in ("/opt/trn_rl_repo",):
    if _p not in sys.path:
        sys.path.append(_p)

import concourse.bass as bass
import concourse.mybir as mybir
from concourse.bass_utils import run_bass_kernel_spmd

B, D = 128, 1024
NCORES = 8
SPC = B // NCORES  # 16 samples per core
KCH = D // 128     # 8 column chunks of 128
LAM = 0.005

FP = mybir.dt.float32
BF = mybir.dt.bfloat16
AF = mybir.ActivationFunctionType
AL = mybir.AluOpType

# consts column layout
C_AMASK = 0
C_EXPAND = 128
C_Z1R = 256
C_Z2R = 384
C_GSEL = 512
C_SELZ1 = 528
C_SELZ2 = 656
C_TOTAL = 784


def build_program():
    nc = bass.Bass("TRN2", debug=False, num_devices=NCORES,
                   detect_race_conditions=False)

    z1_d = nc.dram_tensor("z1", [B, D], FP, kind="ExternalInput")
    z2_d = nc.dram_tensor("z2", [B, D], FP, kind="ExternalInput")
    consts_d = nc.dram_tensor("consts", [128, C_TOTAL], FP, kind="ExternalInput")
    loss_d = nc.dram_tensor("loss", [SPC, 1], FP, kind="ExternalOutput")

    ctx = ExitStack()
    with ctx:
        sem = {n: ctx.enter_context(nc.semaphore(n)) for n in
               ["sca", "dz0", "dz1", "dz2", "dz3", "sscat", "qscat", "dout",
                "spe", "sv", "sa", "sg"]}

        def sb(name, shape, dtype=FP):
            return ctx.enter_context(nc.sbuf_tensor(name, shape, dtype))

        ca = sb("ca", [128, C_TOTAL])
        z = sb("z", [128, 2 * D])
        zb = sb("zb", [128, 2 * D], BF)
        sqb = sb("sqb", [128, 2 * D], BF)
        sumrow = sb("sumrow", [1, 2 * D])
        qrow = sb("qrow", [1, 2 * D])
        sum16 = sb("sum16", [16, 128])
        q16s = sb("q16s", [16, 128])
        q127 = sb("q127", [16, 128])
        t1 = sb("t1", [16, 128])
        var16 = sb("var16", [16, 128])
        std16 = sb("std16", [16, 128])
        A_z = sb("A_z", [16, 128])
        A_zb = sb("A_zb", [16, 128], BF)
        C_zb = sb("C_zb", [16, 128], BF)
        selz1b = sb("selz1b", [16, 128], BF)
        selz2b = sb("selz2b", [16, 128], BF)
        tn1 = sb("tn1", [128, 128])
        z1n = sb("z1n", [128, 128])
        tn2 = sb("tn2", [128, 128])
        z2n = sb("z2n", [128, 128])
        w = sb("w", [128, 128])
        u = sb("u", [128, 128])
        junkP = sb("junkP", [128, 128])
        junkQ = sb("junkQ", [128, 128])
        junkA = sb("junkA", [128, 128])
        junkV = sb("junkV", [128, 128])
        junkG = sb("junkG", [128, 128])
        junkA2 = sb("junkA2", [128, 128])
        junkV2 = sb("junkV2", [128, 128])
        negone = sb("negone", [128, 1])
        ones_b = sb("ones_b", [128, 1], BF)
        acol = sb("acol", [128, 1])
        vcol = sb("vcol", [128, 1])
        a_sb = sb("a_sb", [128, 1])
        colsD = sb("colsD", [128, 8])
        q16sb = sb("q16sb", [16, 8])
        fin = sb("fin", [16, 14])
        loss16 = sb("loss16", [16, 1])

        # PSUM: 4 stat banks (reused by blocks 2,3) + broadcast bank +
        # a-expand bank + group-reduce bank = 7 of 8 banks.
        ps = [ctx.enter_context(nc.psum_tensor(f"ps{i}", [1, 512], FP))
              for i in range(4)]
        ps_s_t = [ps[0], ps[2], ps[0], ps[2]]
        ps_q_t = [ps[1], ps[3], ps[1], ps[3]]
        psBC1 = ctx.enter_context(nc.psum_tensor("psBC1", [128, 256], FP))
        psBC2 = ctx.enter_context(nc.psum_tensor("psBC2", [128, 256], FP))
        q16 = ctx.enter_context(nc.psum_tensor("q16", [16, 8], FP))

        psA1 = psBC1[:, 0:128]
        psC1 = psBC1[:, 128:256]
        psA2 = psBC2[:, 0:128]
        psC2 = psBC2[:, 128:256]

        amask = ca[:, C_AMASK:C_AMASK + 128]
        expand = ca[:, C_EXPAND:C_EXPAND + 128]
        z1r = ca[:, C_Z1R:C_Z1R + 128]
        z2r = ca[:, C_Z2R:C_Z2R + 128]
        gsel = ca[:, C_GSEL:C_GSEL + 16]
        selz1 = ca[0:16, C_SELZ1:C_SELZ1 + 128]
        selz2 = ca[0:16, C_SELZ2:C_SELZ2 + 128]

        blksl = [slice(i * 512, (i + 1) * 512) for i in range(4)]

        with nc.Block() as block:

            @block.sync
            def _(sync):
                sync.dma_start(z[:, blksl[0]], z1_d[:, 0:512]).then_inc(sem["dz0"], 16)
                sync.dma_start(z[:, blksl[1]], z1_d[:, 512:1024]).then_inc(sem["dz1"], 16)
                sync.dma_start(z[:, blksl[2]], z2_d[:, 0:512]).then_inc(sem["dz2"], 16)
                sync.dma_start(z[:, blksl[3]], z2_d[:, 512:1024]).then_inc(sem["dz3"], 16)
                sync.dma_start(ca[:], consts_d[:]).then_inc(sem["sca"], 16)
                sync.wait_ge(sem["sv"], 10)
                sync.dma_start(sum16[:], sumrow[:]).then_inc(sem["sscat"], 16)
                sync.wait_ge(sem["sa"], 6)
                sync.dma_start(q16s[:], qrow[:]).then_inc(sem["qscat"], 16)
                sync.wait_ge(sem["sv"], 51)
                sync.dma_start(loss_d[:], loss16[:]).then_inc(sem["dout"], 16)

            @block.gpsimd
            def _(gp):
                gp.wait_ge(sem["dz2"], 16)
                gp.tensor_tensor(sqb[:, blksl[2]], z[:, blksl[2]], z[:, blksl[2]],
                                 AL.mult).then_inc(sem["sg"])                             # 1
                gp.wait_ge(sem["dz3"], 16)
                gp.tensor_tensor(sqb[:, blksl[3]], z[:, blksl[3]], z[:, blksl[3]],
                                 AL.mult).then_inc(sem["sg"])                             # 2
                gp.wait_ge(sem["dout"], 16)

            @block.scalar
            def _(act):
                act.wait_ge(sem["dz0"], 16)
                act.square(sqb[:, blksl[0]], z[:, blksl[0]]).then_inc(sem["sa"])          # 1
                act.wait_ge(sem["dz1"], 16)
                act.square(sqb[:, blksl[1]], z[:, blksl[1]]).then_inc(sem["sa"])          # 2
                for i in range(4):                                                        # 3-6
                    act.wait_ge(sem["spe"], 2 * i + 2)
                    act.copy(qrow[:, blksl[i]], ps_q_t[i][:]).then_inc(sem["sa"])
                act.wait_ge(sem["sv"], 12)
                act.activation(std16[:], var16[:], AF.Sqrt).then_inc(sem["sa"])           # 7
                act.wait_ge(sem["sv"], 28)
                act.activation(u[:], z2n[:], AF.Square, bias=negone[:],
                               scale=a_sb[:]).then_inc(sem["sa"])                         # 8
                act.wait_ge(sem["spe"], 13)
                act.copy(q16sb[:], q16[:]).then_inc(sem["sa"])                            # 9

            @block.vector
            def _(dve):
                dve.memset(negone[:], -1.0).then_inc(sem["sv"])                           # 1
                dve.memset(ones_b[:], 1.0).then_inc(sem["sv"])                            # 2
                dve.wait_ge(sem["dz0"], 16)
                dve.tensor_copy(zb[:, blksl[0]], z[:, blksl[0]]).then_inc(sem["sv"])      # 3
                dve.wait_ge(sem["dz1"], 16)
                dve.tensor_copy(zb[:, blksl[1]], z[:, blksl[1]]).then_inc(sem["sv"])      # 4
                dve.wait_ge(sem["spe"], 1)
                dve.tensor_copy(sumrow[:, blksl[0]], ps_s_t[0][:]).then_inc(sem["sv"])    # 5
                dve.wait_ge(sem["spe"], 3)
                dve.tensor_copy(sumrow[:, blksl[1]], ps_s_t[1][:]).then_inc(sem["sv"])    # 6
                dve.wait_ge(sem["dz2"], 16)
                dve.tensor_copy(zb[:, blksl[2]], z[:, blksl[2]]).then_inc(sem["sv"])      # 7
                dve.wait_ge(sem["dz3"], 16)
                dve.tensor_copy(zb[:, blksl[3]], z[:, blksl[3]]).then_inc(sem["sv"])      # 8
                dve.wait_ge(sem["spe"], 5)
                dve.tensor_copy(sumrow[:, blksl[2]], ps_s_t[2][:]).then_inc(sem["sv"])    # 9
                dve.wait_ge(sem["spe"], 7)
                dve.tensor_copy(sumrow[:, blksl[3]], ps_s_t[3][:]).then_inc(sem["sv"])    # 10
                dve.wait_ge(sem["sscat"], 16)
                dve.scalar_tensor_tensor(
                    t1[:], sum16[:], 1.0 / (B * (B - 1.0)), sum16[:],
                    op0=AL.mult, op1=AL.mult).then_inc(sem["sv"])                         # 11
                dve.wait_ge(sem["qscat"], 16)
                dve.scalar_tensor_tensor(
                    var16[:], q16s[:], 1.0 / (B - 1.0), t1[:],
                    op0=AL.mult, op1=AL.subtract).then_inc(sem["sv"])                     # 12
                dve.wait_ge(sem["sa"], 7)
                dve.reciprocal(A_z[:], std16[:]).then_inc(sem["sv"])                      # 13
                # selector casts double as spacing before A_z is re-read
                dve.wait_ge(sem["sca"], 16)
                dve.tensor_copy(selz1b[:], selz1).then_inc(sem["sv"])                     # 14
                dve.tensor_copy(selz2b[:], selz2).then_inc(sem["sv"])                     # 15
                dve.tensor_copy(A_zb[:], A_z[:]).then_inc(sem["sv"])                      # 16
                dve.scalar_tensor_tensor(
                    C_zb[:], sum16[:], 1.0 / B, A_z[:],
                    op0=AL.mult, op1=AL.mult).then_inc(sem["sv"])                         # 17
                # normalize z1 after its bank's two matmuls; z2 after the rest
                dve.wait_ge(sem["spe"], 10)
                dve.tensor_tensor(tn1[:], z1r, psA1, AL.mult).then_inc(sem["sv"])         # 18
                dve.tensor_tensor(z1n[:], tn1[:], psC1, AL.subtract).then_inc(sem["sv"])  # 19
                dve.wait_ge(sem["spe"], 12)
                dve.tensor_tensor(tn2[:], z2r, psA2, AL.mult).then_inc(sem["sv"])         # 20
                dve.tensor_tensor(z2n[:], tn2[:], psC2, AL.subtract).then_inc(sem["sv"])  # 21
                dve.scalar_tensor_tensor(
                    w[:], z1n[:], 1.0, z2n[:], op0=AL.bypass, op1=AL.mult,
                    accum_out=colsD[:, 0:1]).then_inc(sem["sv"])                          # 22 R
                dve.scalar_tensor_tensor(
                    junkP[:], w[:], 1.0, w[:], op0=AL.bypass, op1=AL.mult,
                    accum_out=colsD[:, 1:2]).then_inc(sem["sv"])                          # 23 P
                dve.scalar_tensor_tensor(
                    junkA[:], z1n[:], 1.0, amask, op0=AL.bypass, op1=AL.mult,
                    accum_out=acol[:]).then_inc(sem["sv"])                                # 24
                dve.scalar_tensor_tensor(
                    junkV[:], z2n[:], 1.0, amask, op0=AL.bypass, op1=AL.mult,
                    accum_out=vcol[:]).then_inc(sem["sv"])                                # 25
                dve.scalar_tensor_tensor(
                    junkA2[:], z1n[:], 1.0, z1n[:], op0=AL.bypass, op1=AL.mult,
                    accum_out=colsD[:, 6:7]).then_inc(sem["sv"])                          # 26 Sa
                dve.scalar_tensor_tensor(
                    junkV2[:], z2n[:], 1.0, z2n[:], op0=AL.bypass, op1=AL.mult,
                    accum_out=colsD[:, 7:8]).then_inc(sem["sv"])                          # 27 Sv
                dve.stream_shuffle(a_sb[:], acol[:],
                                   [8 * (i // 8) for i in range(32)]).then_inc(sem["sv"])  # 28
                dve.wait_ge(sem["sa"], 8)
                dve.scalar_tensor_tensor(
                    junkQ[:], u[:], 1.0, u[:], op0=AL.bypass, op1=AL.mult,
                    accum_out=colsD[:, 2:3]).then_inc(sem["sv"])                          # 29 Q
                dve.scalar_tensor_tensor(
                    junkG[:], u[:], 1.0, amask, op0=AL.bypass, op1=AL.mult,
                    accum_out=colsD[:, 3:4]).then_inc(sem["sv"])                          # 30 gd
                dve.tensor_tensor(colsD[:, 4:5], acol[:], vcol[:],
                                  AL.mult).then_inc(sem["sv"])                            # 31 d
                dve.tensor_tensor(colsD[:, 5:6], acol[:], acol[:],
                                  AL.mult).then_inc(sem["sv"])                            # 32 a2
                # ---- finals ----
                R_ = q16sb[:, 0:1]
                P_ = q16sb[:, 1:2]
                Q_ = q16sb[:, 2:3]
                gd_ = q16sb[:, 3:4]
                d_ = q16sb[:, 4:5]
                a2_ = q16sb[:, 5:6]
                Sa_ = q16sb[:, 6:7]
                Sv_ = q16sb[:, 7:8]
                d2 = fin[:, 0:1]
                g4 = fin[:, 1:2]
                h = fin[:, 2:3]
                Tp = fin[:, 3:4]
                on1 = fin[:, 4:5]
                on2 = fin[:, 5:6]
                e1 = fin[:, 6:7]
                f1 = fin[:, 7:8]
                f2 = fin[:, 8:9]
                u1 = fin[:, 9:10]
                u2 = fin[:, 10:11]
                off = fin[:, 11:12]
                hm = fin[:, 12:13]
                Tp2 = fin[:, 13:14]
                dve.wait_ge(sem["sa"], 9)
                dve.tensor_tensor(d2, d_, d_, AL.mult).then_inc(sem["sv"])       # 33
                dve.tensor_tensor(g4, gd_, gd_, AL.mult).then_inc(sem["sv"])     # 34
                dve.tensor_scalar_add(hm, gd_, -1.0).then_inc(sem["sv"])         # 35
                dve.scalar_tensor_tensor(
                    Tp, R_, -2.0, P_, op0=AL.mult, op1=AL.add).then_inc(sem["sv"])  # 36
                dve.tensor_tensor(e1, Sa_, a2_, AL.subtract).then_inc(sem["sv"])    # 37
                dve.tensor_tensor(h, hm, hm, AL.mult).then_inc(sem["sv"])        # 38
                dve.tensor_tensor(f2, d2, Q_, AL.add).then_inc(sem["sv"])        # 39
                dve.tensor_scalar_add(Tp2, Tp, float(D)).then_inc(sem["sv"])     # 40
                dve.tensor_tensor(f1, e1, Sv_, AL.mult).then_inc(sem["sv"])      # 41
                dve.tensor_tensor(u2, f2, P_, AL.subtract).then_inc(sem["sv"])   # 42
                dve.scalar_tensor_tensor(
                    on1, gd_, -1.0, Tp2,
                    op0=AL.mult, op1=AL.add).then_inc(sem["sv"])                 # 43
                dve.tensor_tensor(u1, f1, g4, AL.subtract).then_inc(sem["sv"])   # 44
                dve.drain().then_inc(sem["sv"])                                  # 45
                dve.tensor_tensor(on2, on1, h, AL.add).then_inc(sem["sv"])       # 46
                dve.tensor_tensor(off, u1, u2, AL.add).then_inc(sem["sv"])       # 47
                dve.drain().then_inc(sem["sv"])                                  # 48
                dve.scalar_tensor_tensor(
                    loss16[:], off, LAM, on2,
                    op0=AL.mult, op1=AL.add).then_inc(sem["sv"])                 # 49
                dve.drain().then_inc(sem["sv"])                                  # 50
                dve.engine_nop().then_inc(sem["sv"])                             # 51

            @block.tensor
            def _(pe):
                pe.wait_ge(sem["sv"], 3)
                pe.matmul(ps_s_t[0][:], ones_b[:], zb[:, blksl[0]],
                          start=True, stop=True).then_inc(sem["spe"])                     # 1
                pe.wait_ge(sem["sa"], 1)
                pe.matmul(ps_q_t[0][:], ones_b[:], sqb[:, blksl[0]],
                          start=True, stop=True).then_inc(sem["spe"])                     # 2
                pe.wait_ge(sem["sv"], 4)
                pe.matmul(ps_s_t[1][:], ones_b[:], zb[:, blksl[1]],
                          start=True, stop=True).then_inc(sem["spe"])                     # 3
                pe.wait_ge(sem["sa"], 2)
                pe.matmul(ps_q_t[1][:], ones_b[:], sqb[:, blksl[1]],
                          start=True, stop=True).then_inc(sem["spe"])                     # 4
                pe.wait_ge(sem["sv"], 7)   # zb2; WAR s0-copy at sv5
                pe.matmul(ps_s_t[2][:], ones_b[:], zb[:, blksl[2]], start=True,
                          stop=True, skip_group_check=True).then_inc(sem["spe"])          # 5
                pe.wait_ge(sem["sg"], 1)   # sq2
                pe.wait_ge(sem["sa"], 3)   # WAR q0-copy
                pe.matmul(ps_q_t[2][:], ones_b[:], sqb[:, blksl[2]], start=True,
                          stop=True, skip_group_check=True).then_inc(sem["spe"])          # 6
                pe.wait_ge(sem["sv"], 8)   # zb3; WAR s1-copy at sv6
                pe.matmul(ps_s_t[3][:], ones_b[:], zb[:, blksl[3]], start=True,
                          stop=True, skip_group_check=True).then_inc(sem["spe"])          # 7
                pe.wait_ge(sem["sg"], 2)   # sq3
                pe.wait_ge(sem["sa"], 4)   # WAR q1-copy
                pe.matmul(ps_q_t[3][:], ones_b[:], sqb[:, blksl[3]], start=True,
                          stop=True, skip_group_check=True).then_inc(sem["spe"])          # 8
                # broadcasts (bf16): bank A then bank B
                pe.wait_ge(sem["sv"], 16)
                pe.matmul(psA1, selz1b[:], A_zb[:], start=True,
                          stop=True).then_inc(sem["spe"])                                 # 9
                pe.wait_ge(sem["sv"], 17)
                pe.matmul(psC1, selz1b[:], C_zb[:], start=True, stop=True,
                          skip_group_check=True).then_inc(sem["spe"])                     # 10
                pe.matmul(psA2, selz2b[:], A_zb[:], start=True, stop=True,
                          skip_group_check=True).then_inc(sem["spe"])                     # 11
                pe.matmul(psC2, selz2b[:], C_zb[:], start=True, stop=True,
                          skip_group_check=True).then_inc(sem["spe"])                     # 12
                # group reduce
                pe.wait_ge(sem["sv"], 32)
                pe.matmul(q16[:], gsel, colsD[:], start=True,
                          stop=True).then_inc(sem["spe"])                                 # 13

    return nc


def _host_inputs(z1, z2):
    """Per-core input maps (sharding glue)."""
    z1 = np.ascontiguousarray(z1, np.float32)
    z2 = np.ascontiguousarray(z2, np.float32)

    base = np.zeros((128, C_TOTAL), np.float32)
    for m in range(128):
        base[8 * (m // 8), C_EXPAND + m] = 1.0   # expand
        base[m, C_GSEL + m // 8] = 1.0           # gsel
        base[m % 8, C_SELZ1 + m] = 1.0           # selz1
        base[8 + m % 8, C_SELZ2 + m] = 1.0       # selz2

    in_maps = []
    for c in range(NCORES):
        rows = slice(c * SPC, (c + 1) * SPC)
        consts = base.copy()
        consts[:, C_Z1R:C_Z1R + 128] = \
            z1[rows].reshape(SPC, KCH, 128).reshape(128, 128)
        consts[:, C_Z2R:C_Z2R + 128] = \
            z2[rows].reshape(SPC, KCH, 128).reshape(128, 128)
        for s in range(SPC):
            consts[s * 8, C_AMASK + c * SPC + s] = 1.0
        in_maps.append({
            "z1": z1, "z2": z2,
            "consts": np.ascontiguousarray(consts),
        })
    return in_maps


_cached_nc = None


def run(z1, z2, trace=False, **kwargs):
    global _cached_nc
    if _cached_nc is None:
        _cached_nc = build_program()
    in_maps = _host_inputs(z1, z2)
    res = run_bass_kernel_spmd(
        _cached_nc, in_maps, core_ids=list(range(NCORES)), trace=trace, **kwargs)
    out = np.concatenate([res.results[c]["loss"][:, 0] for c in range(NCORES)])
    return out.astype(np.float32), res


def kernel(z1, z2):
    out, _ = run(z1, z2, trace=False)
    return out
